# revision 3
# baseline (speedup 1.0000x reference)
"""Trainium2 Bass kernel for the MANTIS quantum-circuit-loss nn.Module.

Shapes (hardcoded): B=128, L=16, M=32, P=4.  8 NeuronCores, batch-sharded
(16 batch elements per core).

Math
----
Let j = (m, p) flattened (M*P = 128 == partition count) and
    A[b, l, j] = theta[l, j] + scal[p(j)] * input_ds[b, l]
    CA = cos(A), SA = sin(A)                       (ACT Sin + pi/2 bias)

prob term:      amp[b]  = sum_j coef_j prod_l CA[b,l,j]
normalization:  norm[b] = sum_{j,k} coef_j coef_k prod_l cos(A[b,l,j]-A[b,l,k])

Using cos(a-b) = cos a cos b + sin a sin b, norm[b] is the squared norm of a
sum of 128 product states in the 2^16-dim site space.  Split the 16 sites
into two groups of 8; for each group build the 256 branch-product vectors
    U_g[j, T] = prod_{l in g} X_{T_l}[b, l, j],  X_0 = CA, X_1 = SA
by log-doubling (elementwise multiplies, bf16).  Group-0 products run on
DVE, group-1 on the otherwise-idle GpSimd/Pool engine (separate SBUF
ports -> true parallelism).  Then with coef folded into U1 (site 0):
    D_b[T1, T2] = sum_j (c U1)[j, T1] U2[j, T2]    (PE matmul, bf16, K=128)
    norm[b] = sum_{T1,T2} D_b^2
    amp[b]  = D_b[0, 0]
    loss_b  = -(ln(amp^2 + EPS*norm) - ln(norm))   (== -ln(prob/norm + EPS))
The norm square+reduce is split between ACT (Square + accum_out) and DVE
(custom fused sq+reduce DVE op reading PSUM once -- dual-PSUM-operand
instructions are illegal).  Regularization variances use one masked matmul
+ small DVE ops, fully overlapped.  Each core returns [1,2]:
    out[0,0] = -(1/128) * sum_{local b} ln(prob_norm_b + EPS)
    out[0,1] = REG_C*var(coef) + REG_THETA_M*... + REG_THETA_P*...
Host combine: loss = sum_c out_c[0,0] + out_0[0,1].
"""

import math
import os

import numpy as np

import concourse.bacc as bacc
import concourse.bass as bass
import concourse.mybir as mybir
import concourse.tile as tile

B, L, M, P = 128, 16, 32, 4
NCORES = 8
BLOC = B // NCORES  # 16 batch elements per core
J = M * P  # 128
EPS = 1e-20
REG_C = 0.01
REG_THETA_M = 0.01
REG_THETA_P = 0.01

F32 = mybir.dt.float32
BF16 = mybir.dt.bfloat16
MM_DT = mybir.dt.float32r
AF = mybir.ActivationFunctionType
ALU = mybir.AluOpType

CHUNKS = [int(x) for x in os.environ.get("MANTIS_CHUNKS", "2,2,4,4,4").split(",")]
# batch ids whose norm square+reduce runs on DVE (rest on ACT)
DVE_SQ_IDS = [
    int(x) for x in os.environ.get("MANTIS_DVE_SQ", "13,14,15").split(",") if x
]
# square mode for the DVE ids: custom (fused sq+accum), dsq (copy+stt)
SQ_MODE = os.environ.get("MANTIS_SQ_MODE", "custom")
# which group-g L3/L1/L2 builds go to Pool: "g1" (default), "none"
POOL_MODE = os.environ.get("MANTIS_POOL", "g1")
POOL_CH = {int(x) for x in os.environ.get("MANTIS_POOL_CH", "2,3,4").split(",") if x}

# params column layout
PC_THETA = 0  # 16 cols: theta_t[j, l]
PC_COEF = 16  # 1 col
PC_SCAL = 17  # 1 col: pi / 2^(p(j)+1)
PC_DVEC = 18  # 1 col: 1/n for the var terms (rows 0:37)
PC_HALFPI = 19  # 1 col: pi/2 (ACT bias for cos-via-sin)
PC_MASK = 20  # 37 cols: [ones | mask_p(4) | mask_m(32)]
PC_REGW = 57  # 17 cols: reg weights (rows 0:37)
PC_INDS = 74  # BLOC*L cols: input_ds slice (broadcast over partitions)
P_COLS = 74 + BLOC * L  # 330

# FIN column layout ([128, 32])
FC_SQ = 0  # 16: per-b norm partials (per partition = T1 row)
FC_AMP = 16  # 16: per-b amp partials (per partition = j)

_SQ_OP = None


def _register_sq_reduce():
    """Register a custom DVE op: out = sq(in0), accum_out = s0 + sum(out).
    Reads PSUM once -> legal fused square+reduce of the D matrix."""
    global _SQ_OP
    if _SQ_OP is not None:
        return _SQ_OP
    import concourse.dve_ops as dops
    from concourse.dve_spec import Spec, Src0, C0, sq, lower
    from concourse.dve_uop import DveOpSpec
    from operator import add

    name = "SQ_REDUCE_ANT"
    for op in dops.OPS:
        if op.name == name:
            _SQ_OP = op
            return op

    def _ref(in0, in1, s0, s1, imm2):
        b = (in0.astype(np.float32) ** 2).astype(np.float32)
        return b, s0 + b.reshape(b.shape[0], -1).sum(-1, keepdims=True)

    spec = Spec(body=sq(Src0), accum=add, accum_init=C0, reference=_ref)
    row = max(dops._SUB_OPCODE_FOR_NAME.values()) + 1
    shas = {}
    for ver in ("v3", "v4"):
        try:
            tmp = DveOpSpec(name=name, opcode=row, uops=lower(spec, ver=ver), rd1_en=False)
            shas[ver] = tmp.sha(ver)
        except Exception:
            pass
    op = dops.DveOp(name, spec, subdim=False, uops_sha=shas)
    dops.OPS.append(op)
    dops.CUSTOM_DVE_SPECS[name] = spec
    dops._SUB_OPCODE_FOR_NAME[name] = row
    _SQ_OP = op
    return op


def build_params() -> np.ndarray:
    pr = np.zeros((J, P_COLS), dtype=np.float32)
    sf = (np.pi / 2.0 ** (np.arange(P) + 1.0)).astype(np.float32)
    pr[:, PC_SCAL] = np.tile(sf, M)
    # dvec: 1/n divisors for var terms
    pr[0, PC_DVEC] = 1.0 / 128.0
    pr[1:5, PC_DVEC] = 1.0 / 32.0
    pr[5:37, PC_DVEC] = 1.0 / 4.0
    # masks
    pr[:, PC_MASK] = 1.0  # ones
    jj = np.arange(J)
    pr[:, PC_MASK + 1 : PC_MASK + 37] = 0.0
    pr[jj, PC_MASK + 1 + (jj % 4)] = 1.0  # mask_p
    pr[jj, PC_MASK + 5 + (jj // 4)] = 1.0  # mask_m
    # REGW (rows 0:37): weight for each cell of (S^2/n - SS) so that
    # sum(REGW * (S^2/n - SS)) == reg_total.  var = (SS - S^2/n)/(n-1), so
    # weight = -reg_coeff * mean_factor / (n-1).
    pr[0, PC_REGW + 16] = -REG_C / 127.0
    pr[1:5, PC_REGW : PC_REGW + 16] = -REG_THETA_M / 64.0 / 31.0
    pr[5:37, PC_REGW : PC_REGW + 16] = -REG_THETA_P / 512.0 / 3.0
    pr[:, PC_HALFPI] = np.pi / 2.0
    return pr


def build_program():
    """Build the SPMD Bass/Tile program (identical on all 8 cores)."""
    if SQ_MODE == "custom":
        _register_sq_reduce()
    nc = bacc.Bacc(
        "TRN2",
        target_bir_lowering=False,
        debug=False,
        num_devices=NCORES,
    )
    params_d = nc.dram_tensor("params", [J, P_COLS], F32, kind="ExternalInput")
    out_d = nc.dram_tensor("out", [1, 2], F32, kind="ExternalOutput")

    with tile.TileContext(nc) as tc:
        with (
            tc.tile_pool(name="const", bufs=1) as cpool,
            tc.tile_pool(name="work", bufs=1) as wpool,
            tc.tile_pool(name="dps", bufs=5, space=bass.MemorySpace.PSUM) as dpool,
            tc.tile_pool(name="fps", bufs=1, space=bass.MemorySpace.PSUM) as fpool,
        ):
            _emit(nc, tc, cpool, wpool, dpool, fpool, params_d, out_d)
    nc.compile()
    return nc


def _emit(nc, tc, cpool, wpool, dpool, fpool, params_d, out_d):
    params = cpool.tile([J, P_COLS], F32, tag="params")
    nc.sync.dma_start(params[:], params_d[:, :])

    theta_ap = params[:, PC_THETA : PC_THETA + L]
    coef_ap = params[:, PC_COEF : PC_COEF + 1]
    scal_ap = params[:, PC_SCAL : PC_SCAL + 1]
    inds_ap = params[:, PC_INDS : PC_INDS + BLOC * L]

    # --- stage A: ARG[j, (i,l)] = theta[j,l] + scal[j]*inds[i,l]
    arg = wpool.tile([J, BLOC * L], F32, tag="arg")
    in_bc = inds_ap.rearrange("j (i l) -> j i l", i=BLOC, l=L)
    th_bc = theta_ap.unsqueeze(1).broadcast_to([J, BLOC, L])
    arg_v = arg[:].rearrange("j (i l) -> j i l", i=BLOC, l=L)
    nc.vector.scalar_tensor_tensor(
        out=arg_v, in0=in_bc, scalar=scal_ap, in1=th_bc,
        op0=ALU.mult, op1=ALU.add,
    )

    # --- CS[j, (t,i,l)]: t=0 -> cos(A), t=1 -> sin(A); bf16 out
    # cos(A) = sin(pi/2 - A); A in (-1, 2.58) keeps both args in [-pi, pi].
    cs = wpool.tile([J, 2 * BLOC * L], F32, tag="cs")
    nc.scalar.activation(
        cs[:, 0 : BLOC * L], arg[:], AF.Sin,
        bias=params[:, PC_HALFPI : PC_HALFPI + 1], scale=-1.0,
    )
    nc.scalar.activation(cs[:, BLOC * L : 2 * BLOC * L], arg[:], AF.Sin)

    # fold coef into site l=0 (both branches) => every T1 combo of group 0
    # carries exactly one coef_j factor.
    cs_v = cs[:].rearrange("j (t i l) -> j t i l", t=2, i=BLOC, l=L)
    nc.vector.tensor_scalar_mul(cs_v[:, :, :, 0:1], cs_v[:, :, :, 0:1], coef_ap)

    final = wpool.tile([1, 2], F32, tag="final")

    # --- doubling: L1 (site pairs, 4 combos), L2 (quads, 16 combos)
    # group 0 on DVE, group 1 on Pool (own SBUF port; overlaps DVE).
    eng = {0: nc.vector, 1: (nc.gpsimd if POOL_MODE == "g1" else nc.vector)}
    l1 = [wpool.tile([J, BLOC * 16], F32, tag=f"l1_{g}", name=f"l1_{g}") for g in range(2)]
    l2 = [wpool.tile([J, BLOC * 32], F32, tag=f"l2_{g}", name=f"l2_{g}") for g in range(2)]
    for g in range(2):
        lo = g * 8  # first site of the group
        o1all = l1[g][:].rearrange(
            "j (i s t1 t2) -> j i s t1 t2", i=BLOC, s=4, t1=2, t2=2
        )
        for t1 in range(2):
            in1 = (
                cs_v[:, t1, :, lo : lo + 8 : 2]
                .unsqueeze(3)
                .broadcast_to([J, BLOC, 4, 2])
            )
            in2 = cs_v[:, :, :, lo + 1 : lo + 8 : 2].transpose([0, 2, 3, 1])
            o1 = o1all[:, :, :, t1, :]
            eng[g].tensor_tensor(out=o1, in0=in1, in1=in2, op=ALU.mult)
        l1v = l1[g][:].rearrange("j (i s c) -> j i s c", i=BLOC, s=4, c=4)
        o2all = l2[g][:].rearrange(
            "j (i d q1 q2) -> j i d q1 q2", i=BLOC, d=2, q1=4, q2=4
        )
        for d in range(2):
            in1 = l1v[:, :, 2 * d, :].unsqueeze(3).broadcast_to([J, BLOC, 4, 4])
            in2 = l1v[:, :, 2 * d + 1, :].unsqueeze(2).broadcast_to([J, BLOC, 4, 4])
            o2 = o2all[:, :, d, :, :]
            eng[g].tensor_tensor(out=o2, in0=in1, in1=in2, op=ALU.mult)

    # =====================================================================
    # regularization path -- depends only on params; runs early, fully
    # overlapped with the heavy math.
    fin_r = wpool.tile([J, 34], F32, tag="fin_r")
    nc.vector.tensor_copy(fin_r[:, 0:17], params[:, 0:17])
    nc.vector.tensor_tensor(out=fin_r[:, 17:34], in0=params[:, 0:17],
                        in1=params[:, 0:17], op=ALU.mult)
    fout_r = fpool.tile([37, 34], F32, tag="fout_r")
    nc.tensor.matmul(fout_r[:], params[:, PC_MASK : PC_MASK + 37], fin_r[:])
    ss_part = fout_r[0:37, 17:34]
    sv = wpool.tile([37, 17], F32, tag="sv")
    nc.vector.tensor_copy(sv[:], fout_r[0:37, 0:17])
    v1 = wpool.tile([37, 17], F32, tag="v1")
    nc.vector.tensor_tensor(out=v1[:], in0=sv[:], in1=sv[:], op=ALU.mult)
    v2 = wpool.tile([37, 17], F32, tag="v2")
    nc.vector.scalar_tensor_tensor(
        out=v2[:], in0=v1[:],
        scalar=params[0:37, PC_DVEC : PC_DVEC + 1],
        in1=ss_part, op0=ALU.mult, op1=ALU.subtract,
    )
    v3 = wpool.tile([37, 17], F32, tag="v3")
    nc.vector.tensor_tensor(
        out=v3[:], in0=v2[:],
        in1=params[0:37, PC_REGW : PC_REGW + 17], op=ALU.mult,
    )
    v4 = wpool.tile([37, 17], F32, tag="v4")
    v5 = wpool.tile([37, 1], F32, tag="v5")
    nc.vector.tensor_scalar(
        out=v4[:], in0=v3[:], scalar1=1.0, scalar2=None,
        op0=ALU.mult, op1=ALU.add, accum_out=v5[:],
    )
    rt = fpool.tile([1, 1], F32, tag="rt")
    nc.tensor.matmul(rt[:], params[0:37, PC_MASK : PC_MASK + 1], v5[:])
    nc.vector.tensor_copy(final[0:1, 1:2], rt[:])
    # =====================================================================

    # --- L3 chunked by batch; per-chunk U tiles so PE/ACT pipeline per chunk
    fin = wpool.tile([J, 32], F32, tag="fin")  # 0:16 sumsq, 16:32 amp
    chunk_sizes = CHUNKS
    assert sum(chunk_sizes) == BLOC
    sq_dve = set(DVE_SQ_IDS)
    sqjunk = wpool.tile([J, 512], BF16, tag="sqjunk")  # junk out, bf16 halves write
    with tc.tile_pool(name="dsqp", bufs=2) as spool:
        i0 = 0
        for c, csz in enumerate(chunk_sizes):
            cw = csz * 256
            uc = [
                wpool.tile([J, cw], MM_DT, tag=f"u_{g}_{c}", name=f"u_{g}_{c}")
                for g in range(2)
            ]
            for g in range(2):
                l2v = l2[g][:].rearrange(
                    "j (i d c16) -> j i d c16", i=BLOC, d=2, c16=16
                )
                in1 = (
                    l2v[:, i0 : i0 + csz, 0, :]
                    .unsqueeze(3)
                    .broadcast_to([J, csz, 16, 16])
                )
                in2 = (
                    l2v[:, i0 : i0 + csz, 1, :]
                    .unsqueeze(2)
                    .broadcast_to([J, csz, 16, 16])
                )
                ov = uc[g][:].rearrange(
                    "j (i u1 u2) -> j i u1 u2", i=csz, u1=16, u2=16
                )
                e = eng[g] if (g == 1 and c in POOL_CH) else nc.vector
                e.tensor_tensor(out=ov, in0=in1, in1=in2, op=ALU.mult)

            # amp partials for this chunk: fin[:, 16+i] = cU1[j,i,0]*U2[j,i,0]
            u1v = uc[0][:].rearrange("j (i t) -> j i t", i=csz, t=256)
            u2v = uc[1][:].rearrange("j (i t) -> j i t", i=csz, t=256)
            nc.vector.tensor_tensor(
                out=fin[:, 16 + i0 : 16 + i0 + csz],
                in0=u1v[:, :, 0], in1=u2v[:, :, 0], op=ALU.mult,
            )

            # D matmuls + square/accum for this chunk's batch elements
            for k in range(csz):
                i = i0 + k
                dt = dpool.tile([J, 512], F32, tag="D")
                rhs = uc[1][:, k * 256 : (k + 1) * 256]
                for h in range(2):
                    lhsT = uc[0][:, k * 256 + h * 128 : k * 256 + (h + 1) * 128]
                    nc.tensor.matmul(dt[:, h * 256 : (h + 1) * 256], lhsT, rhs)
                if i in sq_dve and SQ_MODE == "custom":
                    nc.vector._custom_dve(
                        _SQ_OP, out=sqjunk[:], in0=dt[:],
                        accum_out=fin[:, i : i + 1],
                    )
                elif i in sq_dve:
                    dsq = spool.tile([J, 512], F32, tag="dsq", name="dsq")
                    nc.vector.tensor_copy(dsq[:], dt[:])
                    nc.vector.scalar_tensor_tensor(
                        out=dsq[:], in0=dsq[:], scalar=1.0, in1=dsq[:],
                        op0=ALU.mult, op1=ALU.mult,
                        accum_out=fin[:, i : i + 1],
                    )
                else:
                    nc.scalar.activation(
                        dt[:], dt[:], AF.Square,
                        accum_out=fin[:, i : i + 1],
                    )
            i0 += csz

    # --- loss tail: one ones-matmul + short scalar chain
    fout = fpool.tile([1, 32], F32, tag="fout")
    nc.tensor.matmul(fout[:], params[:, PC_MASK : PC_MASK + 1], fin[:])
    # g0 = [r(16) | amp(16)] in SBUF
    g0 = wpool.tile([1, 32], F32, tag="g0")
    nc.vector.tensor_copy(g0[:], fout[0:1, 0:32])
    m2 = wpool.tile([1, BLOC], F32, tag="m2")
    nc.vector.tensor_tensor(
        out=m2[:], in0=g0[0:1, 16:32], in1=g0[0:1, 16:32], op=ALU.mult
    )
    # tt = [amp^2 + EPS*r (16) | r (16)]; one Ln over 32 lanes
    tt = wpool.tile([1, 32], F32, tag="tt")
    nc.vector.scalar_tensor_tensor(
        out=tt[0:1, 0:16], in0=g0[0:1, 0:16], scalar=EPS, in1=m2[:],
        op0=ALU.mult, op1=ALU.add,
    )
    nc.vector.tensor_copy(tt[0:1, 16:32], g0[0:1, 0:16])
    lno = wpool.tile([1, 32], F32, tag="lno")
    nc.scalar.activation(lno[:], tt[:], AF.Ln)
    diff = wpool.tile([1, BLOC], F32, tag="diff")
    nc.vector.tensor_tensor(
        out=diff[:], in0=lno[0:1, 0:16], in1=lno[0:1, 16:32], op=ALU.subtract
    )
    scr6 = wpool.tile([1, BLOC], F32, tag="scr6")
    nc.vector.tensor_scalar(
        out=scr6[:], in0=diff[:], scalar1=-1.0 / float(B), scalar2=None,
        op0=ALU.mult, op1=ALU.add, accum_out=final[0:1, 0:1],
    )

    nc.sync.dma_start(out_d[:, :], final[:])


def make_in_maps(input_ds, theta, coef):
    input_ds = np.asarray(input_ds, dtype=np.float32)
    theta = np.asarray(theta, dtype=np.float32)
    coef = np.asarray(coef, dtype=np.float32)
    pr = build_params()
    pr[:, PC_THETA : PC_THETA + L] = theta.transpose(1, 2, 0).reshape(J, L)
    pr[:, PC_COEF] = coef.reshape(J)
    in_maps = []
    for c in range(NCORES):
        prc = pr.copy()
        sl = input_ds[c * BLOC : (c + 1) * BLOC, :].reshape(1, BLOC * L)
        prc[:, PC_INDS:] = np.broadcast_to(sl, (J, BLOC * L))
        in_maps.append({"params": prc})
    return in_maps


_NC_CACHE = None


def _get_program():
    global _NC_CACHE
    if _NC_CACHE is None:
        _NC_CACHE = build_program()
    return _NC_CACHE


def combine_outputs(results):
    loss = 0.0
    for c in range(NCORES):
        loss += float(results[c]["out"][0, 0])
    loss += float(results[0]["out"][0, 1])
    return np.float32(loss)


def kernel(input_ds, theta, coef):
    from concourse.bass_utils import run_bass_kernel_spmd

    nc = _get_program()
    in_maps = make_in_maps(input_ds, theta, coef)
    res = run_bass_kernel_spmd(nc, in_maps, core_ids=list(range(NCORES)))
    return combine_outputs(res.results)


# revision 4
# speedup vs baseline: 1.2007x; 1.2007x over previous
"""Trainium2 Bass kernel for the MANTIS quantum-circuit-loss nn.Module.

Shapes (hardcoded): B=128, L=16, M=32, P=4.  8 NeuronCores, batch-sharded
(16 batch elements per core).

Math
----
Let j = (m, p) flattened (M*P = 128 == partition count) and
    A[b, l, j] = theta[l, j] + scal[p(j)] * input_ds[b, l]
    CA = cos(A), SA = sin(A)                       (ACT Sin + pi/2 bias)

prob term:      amp[b]  = sum_j coef_j prod_l CA[b,l,j]
normalization:  norm[b] = sum_{j,k} coef_j coef_k prod_l cos(A[b,l,j]-A[b,l,k])

Using cos(a-b) = cos a cos b + sin a sin b, norm[b] is the squared norm of a
sum of 128 product states in the 2^16-dim site space.  Split the 16 sites
into two groups of 8; for each group build the 256 branch-product vectors
    U_g[j, T] = prod_{l in g} X_{T_l}[b, l, j],  X_0 = CA, X_1 = SA
by log-doubling (elementwise multiplies, bf16).  Group-0 products run on
DVE, group-1 on the otherwise-idle GpSimd/Pool engine (separate SBUF
ports -> true parallelism).  Then with coef folded into U1 (site 0):
    D_b[T1, T2] = sum_j (c U1)[j, T1] U2[j, T2]    (PE matmul, bf16, K=128)
    norm[b] = sum_{T1,T2} D_b^2
    amp[b]  = D_b[0, 0]
    loss_b  = -(ln(amp^2 + EPS*norm) - ln(norm))   (== -ln(prob/norm + EPS))
The norm square+reduce is split between ACT (Square + accum_out) and DVE
(custom fused sq+reduce DVE op reading PSUM once -- dual-PSUM-operand
instructions are illegal).  Regularization variances use one masked matmul
+ small DVE ops, fully overlapped.  Each core returns [1,2]:
    out[0,0] = -(1/128) * sum_{local b} ln(prob_norm_b + EPS)
    out[0,1] = REG_C*var(coef) + REG_THETA_M*... + REG_THETA_P*...
Host combine: loss = sum_c out_c[0,0] + out_0[0,1].
"""

import math
import os

import numpy as np

import concourse.bacc as bacc
import concourse.bass as bass
import concourse.mybir as mybir
import concourse.tile as tile

B, L, M, P = 128, 16, 32, 4
NCORES = 8
BLOC = B // NCORES  # 16 batch elements per core
J = M * P  # 128
EPS = 1e-20
REG_C = 0.01
REG_THETA_M = 0.01
REG_THETA_P = 0.01

F32 = mybir.dt.float32
BF16 = mybir.dt.bfloat16
MM_DT = mybir.dt.float32r
AF = mybir.ActivationFunctionType
ALU = mybir.AluOpType

CHUNKS = [int(x) for x in os.environ.get("MANTIS_CHUNKS", "2,2,4,4,4").split(",")]
# batch ids whose norm square+reduce runs on DVE (rest on ACT)
DVE_SQ_IDS = [
    int(x) for x in os.environ.get("MANTIS_DVE_SQ", "13,14,15").split(",") if x
]
# square mode for the DVE ids: custom (fused sq+accum), dsq (copy+stt)
SQ_MODE = os.environ.get("MANTIS_SQ_MODE", "custom")
# which group-g L3/L1/L2 builds go to Pool: "g1" (default), "none"
POOL_MODE = os.environ.get("MANTIS_POOL", "none")
POOL_CH = {int(x) for x in os.environ.get("MANTIS_POOL_CH", "").split(",") if x}

# params column layout
PC_THETA = 0  # 16 cols: theta_t[j, l]
PC_COEF = 16  # 1 col
PC_SCAL = 17  # 1 col: pi / 2^(p(j)+1)
PC_DVEC = 18  # 1 col: 1/n for the var terms (rows 0:37)
PC_HALFPI = 19  # 1 col: pi/2 (ACT bias for cos-via-sin)
PC_MASK = 20  # 37 cols: [ones | mask_p(4) | mask_m(32)]
PC_REGW = 57  # 17 cols: reg weights (rows 0:37)
PC_INDS = 74  # BLOC*L cols: input_ds slice (broadcast over partitions)
P_COLS = 74 + BLOC * L  # 330

# FIN column layout ([128, 32])
FC_SQ = 0  # 16: per-b norm partials (per partition = T1 row)
FC_AMP = 16  # 16: per-b amp partials (per partition = j)

_SQ_OP = None


def _register_sq_reduce():
    """Register a custom DVE op: out = sq(in0), accum_out = s0 + sum(out).
    Reads PSUM once -> legal fused square+reduce of the D matrix."""
    global _SQ_OP
    if _SQ_OP is not None:
        return _SQ_OP
    import concourse.dve_ops as dops
    from concourse.dve_spec import Spec, Src0, C0, sq, lower
    from concourse.dve_uop import DveOpSpec
    from operator import add

    name = "SQ_REDUCE_ANT"
    for op in dops.OPS:
        if op.name == name:
            _SQ_OP = op
            return op

    def _ref(in0, in1, s0, s1, imm2):
        b = (in0.astype(np.float32) ** 2).astype(np.float32)
        return b, s0 + b.reshape(b.shape[0], -1).sum(-1, keepdims=True)

    spec = Spec(body=sq(Src0), accum=add, accum_init=C0, reference=_ref)
    row = max(dops._SUB_OPCODE_FOR_NAME.values()) + 1
    shas = {}
    for ver in ("v3", "v4"):
        try:
            tmp = DveOpSpec(name=name, opcode=row, uops=lower(spec, ver=ver), rd1_en=False)
            shas[ver] = tmp.sha(ver)
        except Exception:
            pass
    op = dops.DveOp(name, spec, subdim=False, uops_sha=shas)
    dops.OPS.append(op)
    dops.CUSTOM_DVE_SPECS[name] = spec
    dops._SUB_OPCODE_FOR_NAME[name] = row
    _SQ_OP = op
    return op


def build_params() -> np.ndarray:
    pr = np.zeros((J, P_COLS), dtype=np.float32)
    sf = (np.pi / 2.0 ** (np.arange(P) + 1.0)).astype(np.float32)
    pr[:, PC_SCAL] = np.tile(sf, M)
    # dvec: 1/n divisors for var terms
    pr[0, PC_DVEC] = 1.0 / 128.0
    pr[1:5, PC_DVEC] = 1.0 / 32.0
    pr[5:37, PC_DVEC] = 1.0 / 4.0
    # masks
    pr[:, PC_MASK] = 1.0  # ones
    jj = np.arange(J)
    pr[:, PC_MASK + 1 : PC_MASK + 37] = 0.0
    pr[jj, PC_MASK + 1 + (jj % 4)] = 1.0  # mask_p
    pr[jj, PC_MASK + 5 + (jj // 4)] = 1.0  # mask_m
    # REGW (rows 0:37): weight for each cell of (S^2/n - SS) so that
    # sum(REGW * (S^2/n - SS)) == reg_total.  var = (SS - S^2/n)/(n-1), so
    # weight = -reg_coeff * mean_factor / (n-1).
    pr[0, PC_REGW + 16] = -REG_C / 127.0
    pr[1:5, PC_REGW : PC_REGW + 16] = -REG_THETA_M / 64.0 / 31.0
    pr[5:37, PC_REGW : PC_REGW + 16] = -REG_THETA_P / 512.0 / 3.0
    pr[:, PC_HALFPI] = np.pi / 2.0
    return pr


def build_program():
    """Build the SPMD Bass/Tile program (identical on all 8 cores)."""
    if SQ_MODE == "custom":
        _register_sq_reduce()
    nc = bacc.Bacc(
        "TRN2",
        target_bir_lowering=False,
        debug=False,
        num_devices=NCORES,
    )
    params_d = nc.dram_tensor("params", [J, P_COLS], F32, kind="ExternalInput")
    out_d = nc.dram_tensor("out", [1, 2], F32, kind="ExternalOutput")

    with tile.TileContext(nc) as tc:
        with (
            tc.tile_pool(name="const", bufs=1) as cpool,
            tc.tile_pool(name="work", bufs=1) as wpool,
            tc.tile_pool(name="dps", bufs=5, space=bass.MemorySpace.PSUM) as dpool,
            tc.tile_pool(name="fps", bufs=1, space=bass.MemorySpace.PSUM) as fpool,
        ):
            _emit(nc, tc, cpool, wpool, dpool, fpool, params_d, out_d)
    nc.compile()
    return nc


def _emit(nc, tc, cpool, wpool, dpool, fpool, params_d, out_d):
    params = cpool.tile([J, P_COLS], F32, tag="params")
    nc.sync.dma_start(params[:], params_d[:, :])

    theta_ap = params[:, PC_THETA : PC_THETA + L]
    coef_ap = params[:, PC_COEF : PC_COEF + 1]
    scal_ap = params[:, PC_SCAL : PC_SCAL + 1]
    inds_ap = params[:, PC_INDS : PC_INDS + BLOC * L]

    # --- stage A: ARG[j, (i,l)] = theta[j,l] + scal[j]*inds[i,l]
    arg = wpool.tile([J, BLOC * L], F32, tag="arg")
    in_bc = inds_ap.rearrange("j (i l) -> j i l", i=BLOC, l=L)
    th_bc = theta_ap.unsqueeze(1).broadcast_to([J, BLOC, L])
    arg_v = arg[:].rearrange("j (i l) -> j i l", i=BLOC, l=L)
    nc.vector.scalar_tensor_tensor(
        out=arg_v, in0=in_bc, scalar=scal_ap, in1=th_bc,
        op0=ALU.mult, op1=ALU.add,
    )

    # --- CS[j, (t,i,l)]: t=0 -> cos(A), t=1 -> sin(A); bf16 out
    # cos(A) = sin(pi/2 - A); A in (-1, 2.58) keeps both args in [-pi, pi].
    cs = wpool.tile([J, 2 * BLOC * L], F32, tag="cs")
    nc.scalar.activation(
        cs[:, 0 : BLOC * L], arg[:], AF.Sin,
        bias=params[:, PC_HALFPI : PC_HALFPI + 1], scale=-1.0,
    )
    nc.scalar.activation(cs[:, BLOC * L : 2 * BLOC * L], arg[:], AF.Sin)

    # fold coef into site l=0 (both branches) => every T1 combo of group 0
    # carries exactly one coef_j factor.
    cs_v = cs[:].rearrange("j (t i l) -> j t i l", t=2, i=BLOC, l=L)
    nc.vector.tensor_scalar_mul(cs_v[:, :, :, 0:1], cs_v[:, :, :, 0:1], coef_ap)

    final = wpool.tile([1, 2], F32, tag="final")

    # --- doubling: L1 (site pairs, 4 combos), L2 (quads, 16 combos)
    # group 0 on DVE, group 1 on Pool (own SBUF port; overlaps DVE).
    eng = {0: nc.vector, 1: (nc.gpsimd if POOL_MODE == "g1" else nc.vector)}
    l1 = [wpool.tile([J, BLOC * 16], F32, tag=f"l1_{g}", name=f"l1_{g}") for g in range(2)]
    l2 = [wpool.tile([J, BLOC * 32], F32, tag=f"l2_{g}", name=f"l2_{g}") for g in range(2)]
    for g in range(2):
        lo = g * 8  # first site of the group
        o1all = l1[g][:].rearrange(
            "j (i s t1 t2) -> j i s t1 t2", i=BLOC, s=4, t1=2, t2=2
        )
        for t1 in range(2):
            in1 = (
                cs_v[:, t1, :, lo : lo + 8 : 2]
                .unsqueeze(3)
                .broadcast_to([J, BLOC, 4, 2])
            )
            in2 = cs_v[:, :, :, lo + 1 : lo + 8 : 2].transpose([0, 2, 3, 1])
            o1 = o1all[:, :, :, t1, :]
            eng[g].tensor_tensor(out=o1, in0=in1, in1=in2, op=ALU.mult)
        l1v = l1[g][:].rearrange("j (i s c) -> j i s c", i=BLOC, s=4, c=4)
        o2all = l2[g][:].rearrange(
            "j (i d q1 q2) -> j i d q1 q2", i=BLOC, d=2, q1=4, q2=4
        )
        for d in range(2):
            in1 = l1v[:, :, 2 * d, :].unsqueeze(3).broadcast_to([J, BLOC, 4, 4])
            in2 = l1v[:, :, 2 * d + 1, :].unsqueeze(2).broadcast_to([J, BLOC, 4, 4])
            o2 = o2all[:, :, d, :, :]
            eng[g].tensor_tensor(out=o2, in0=in1, in1=in2, op=ALU.mult)

    # =====================================================================
    # regularization path -- depends only on params; runs early, fully
    # overlapped with the heavy math.
    fin_r = wpool.tile([J, 34], F32, tag="fin_r")
    nc.vector.tensor_copy(fin_r[:, 0:17], params[:, 0:17])
    nc.vector.tensor_tensor(out=fin_r[:, 17:34], in0=params[:, 0:17],
                        in1=params[:, 0:17], op=ALU.mult)
    fout_r = fpool.tile([37, 34], F32, tag="fout_r")
    nc.tensor.matmul(fout_r[:], params[:, PC_MASK : PC_MASK + 37], fin_r[:])
    ss_part = fout_r[0:37, 17:34]
    sv = wpool.tile([37, 17], F32, tag="sv")
    nc.vector.tensor_copy(sv[:], fout_r[0:37, 0:17])
    v1 = wpool.tile([37, 17], F32, tag="v1")
    nc.vector.tensor_tensor(out=v1[:], in0=sv[:], in1=sv[:], op=ALU.mult)
    v2 = wpool.tile([37, 17], F32, tag="v2")
    nc.vector.scalar_tensor_tensor(
        out=v2[:], in0=v1[:],
        scalar=params[0:37, PC_DVEC : PC_DVEC + 1],
        in1=ss_part, op0=ALU.mult, op1=ALU.subtract,
    )
    v3 = wpool.tile([37, 17], F32, tag="v3")
    nc.vector.tensor_tensor(
        out=v3[:], in0=v2[:],
        in1=params[0:37, PC_REGW : PC_REGW + 17], op=ALU.mult,
    )
    v4 = wpool.tile([37, 17], F32, tag="v4")
    v5 = wpool.tile([37, 1], F32, tag="v5")
    nc.vector.tensor_scalar(
        out=v4[:], in0=v3[:], scalar1=1.0, scalar2=None,
        op0=ALU.mult, op1=ALU.add, accum_out=v5[:],
    )
    rt = fpool.tile([1, 1], F32, tag="rt")
    nc.tensor.matmul(rt[:], params[0:37, PC_MASK : PC_MASK + 1], v5[:])
    nc.vector.tensor_copy(final[0:1, 1:2], rt[:])
    # =====================================================================

    # --- L3 chunked by batch; per-chunk U tiles so PE/ACT pipeline per chunk
    fin = wpool.tile([J, 32], F32, tag="fin")  # 0:16 sumsq, 16:32 amp
    chunk_sizes = CHUNKS
    assert sum(chunk_sizes) == BLOC
    sq_dve = set(DVE_SQ_IDS)
    sqjunk = wpool.tile([J, 512], BF16, tag="sqjunk")  # junk out, bf16 halves write
    with tc.tile_pool(name="dsqp", bufs=2) as spool:
        i0 = 0
        for c, csz in enumerate(chunk_sizes):
            cw = csz * 256
            uc = [
                wpool.tile([J, cw], MM_DT, tag=f"u_{g}_{c}", name=f"u_{g}_{c}")
                for g in range(2)
            ]
            for g in range(2):
                l2v = l2[g][:].rearrange(
                    "j (i d c16) -> j i d c16", i=BLOC, d=2, c16=16
                )
                in1 = (
                    l2v[:, i0 : i0 + csz, 0, :]
                    .unsqueeze(3)
                    .broadcast_to([J, csz, 16, 16])
                )
                in2 = (
                    l2v[:, i0 : i0 + csz, 1, :]
                    .unsqueeze(2)
                    .broadcast_to([J, csz, 16, 16])
                )
                ov = uc[g][:].rearrange(
                    "j (i u1 u2) -> j i u1 u2", i=csz, u1=16, u2=16
                )
                e = eng[g] if (g == 1 and c in POOL_CH) else nc.vector
                e.tensor_tensor(out=ov, in0=in1, in1=in2, op=ALU.mult)

            # amp partials for this chunk: fin[:, 16+i] = cU1[j,i,0]*U2[j,i,0]
            u1v = uc[0][:].rearrange("j (i t) -> j i t", i=csz, t=256)
            u2v = uc[1][:].rearrange("j (i t) -> j i t", i=csz, t=256)
            nc.vector.tensor_tensor(
                out=fin[:, 16 + i0 : 16 + i0 + csz],
                in0=u1v[:, :, 0], in1=u2v[:, :, 0], op=ALU.mult,
            )

            # D matmuls + square/accum for this chunk's batch elements
            for k in range(csz):
                i = i0 + k
                dt = dpool.tile([J, 512], F32, tag="D")
                rhs = uc[1][:, k * 256 : (k + 1) * 256]
                for h in range(2):
                    lhsT = uc[0][:, k * 256 + h * 128 : k * 256 + (h + 1) * 128]
                    nc.tensor.matmul(dt[:, h * 256 : (h + 1) * 256], lhsT, rhs)
                if i in sq_dve and SQ_MODE == "custom":
                    nc.vector._custom_dve(
                        _SQ_OP, out=sqjunk[:], in0=dt[:],
                        accum_out=fin[:, i : i + 1],
                    )
                elif i in sq_dve:
                    dsq = spool.tile([J, 512], F32, tag="dsq", name="dsq")
                    nc.vector.tensor_copy(dsq[:], dt[:])
                    nc.vector.scalar_tensor_tensor(
                        out=dsq[:], in0=dsq[:], scalar=1.0, in1=dsq[:],
                        op0=ALU.mult, op1=ALU.mult,
                        accum_out=fin[:, i : i + 1],
                    )
                else:
                    nc.scalar.activation(
                        dt[:], dt[:], AF.Square,
                        accum_out=fin[:, i : i + 1],
                    )
            i0 += csz

    # --- loss tail: one ones-matmul + short scalar chain
    fout = fpool.tile([1, 32], F32, tag="fout")
    nc.tensor.matmul(fout[:], params[:, PC_MASK : PC_MASK + 1], fin[:])
    # g0 = [r(16) | amp(16)] in SBUF
    g0 = wpool.tile([1, 32], F32, tag="g0")
    nc.vector.tensor_copy(g0[:], fout[0:1, 0:32])
    m2 = wpool.tile([1, BLOC], F32, tag="m2")
    nc.vector.tensor_tensor(
        out=m2[:], in0=g0[0:1, 16:32], in1=g0[0:1, 16:32], op=ALU.mult
    )
    # tt = [amp^2 + EPS*r (16) | r (16)]; one Ln over 32 lanes
    tt = wpool.tile([1, 32], F32, tag="tt")
    nc.vector.scalar_tensor_tensor(
        out=tt[0:1, 0:16], in0=g0[0:1, 0:16], scalar=EPS, in1=m2[:],
        op0=ALU.mult, op1=ALU.add,
    )
    nc.vector.tensor_copy(tt[0:1, 16:32], g0[0:1, 0:16])
    lno = wpool.tile([1, 32], F32, tag="lno")
    nc.scalar.activation(lno[:], tt[:], AF.Ln)
    diff = wpool.tile([1, BLOC], F32, tag="diff")
    nc.vector.tensor_tensor(
        out=diff[:], in0=lno[0:1, 0:16], in1=lno[0:1, 16:32], op=ALU.subtract
    )
    scr6 = wpool.tile([1, BLOC], F32, tag="scr6")
    nc.vector.tensor_scalar(
        out=scr6[:], in0=diff[:], scalar1=-1.0 / float(B), scalar2=None,
        op0=ALU.mult, op1=ALU.add, accum_out=final[0:1, 0:1],
    )

    nc.sync.dma_start(out_d[:, :], final[:])


def make_in_maps(input_ds, theta, coef):
    input_ds = np.asarray(input_ds, dtype=np.float32)
    theta = np.asarray(theta, dtype=np.float32)
    coef = np.asarray(coef, dtype=np.float32)
    pr = build_params()
    pr[:, PC_THETA : PC_THETA + L] = theta.transpose(1, 2, 0).reshape(J, L)
    pr[:, PC_COEF] = coef.reshape(J)
    in_maps = []
    for c in range(NCORES):
        prc = pr.copy()
        sl = input_ds[c * BLOC : (c + 1) * BLOC, :].reshape(1, BLOC * L)
        prc[:, PC_INDS:] = np.broadcast_to(sl, (J, BLOC * L))
        in_maps.append({"params": prc})
    return in_maps


_NC_CACHE = None


def _get_program():
    global _NC_CACHE
    if _NC_CACHE is None:
        _NC_CACHE = build_program()
    return _NC_CACHE


def combine_outputs(results):
    loss = 0.0
    for c in range(NCORES):
        loss += float(results[c]["out"][0, 0])
    loss += float(results[0]["out"][0, 1])
    return np.float32(loss)


def kernel(input_ds, theta, coef):
    from concourse.bass_utils import run_bass_kernel_spmd

    nc = _get_program()
    in_maps = make_in_maps(input_ds, theta, coef)
    res = run_bass_kernel_spmd(nc, in_maps, core_ids=list(range(NCORES)))
    return combine_outputs(res.results)


# revision 5
# speedup vs baseline: 1.2146x; 1.0116x over previous
"""Trainium2 Bass kernel for the MANTIS quantum-circuit-loss nn.Module.

Shapes (hardcoded): B=128, L=16, M=32, P=4.  8 NeuronCores, batch-sharded
(16 batch elements per core).

Math
----
Let j = (m, p) flattened (M*P = 128 == partition count) and
    A[b, l, j] = theta[l, j] + scal[p(j)] * input_ds[b, l]
    CA = cos(A), SA = sin(A)                       (ACT Sin + pi/2 bias)

prob term:      amp[b]  = sum_j coef_j prod_l CA[b,l,j]
normalization:  norm[b] = sum_{j,k} coef_j coef_k prod_l cos(A[b,l,j]-A[b,l,k])

Using cos(a-b) = cos a cos b + sin a sin b, norm[b] is the squared norm of a
sum of 128 product states in the 2^16-dim site space.  Split the 16 sites
into two groups of 8; for each group build the 256 branch-product vectors
    U_g[j, T] = prod_{l in g} X_{T_l}[b, l, j],  X_0 = CA, X_1 = SA
by log-doubling (elementwise multiplies, bf16).  Group-0 products run on
DVE, group-1 on the otherwise-idle GpSimd/Pool engine (separate SBUF
ports -> true parallelism).  Then with coef folded into U1 (site 0):
    D_b[T1, T2] = sum_j (c U1)[j, T1] U2[j, T2]    (PE matmul, bf16, K=128)
    norm[b] = sum_{T1,T2} D_b^2
    amp[b]  = D_b[0, 0]
    loss_b  = -(ln(amp^2 + EPS*norm) - ln(norm))   (== -ln(prob/norm + EPS))
The norm square+reduce is split between ACT (Square + accum_out) and DVE
(custom fused sq+reduce DVE op reading PSUM once -- dual-PSUM-operand
instructions are illegal).  Regularization variances use one masked matmul
+ small DVE ops, fully overlapped.  Each core returns [1,2]:
    out[0,0] = -(1/128) * sum_{local b} ln(prob_norm_b + EPS)
    out[0,1] = REG_C*var(coef) + REG_THETA_M*... + REG_THETA_P*...
Host combine: loss = sum_c out_c[0,0] + out_0[0,1].
"""

import math
import os

import numpy as np

import concourse.bacc as bacc
import concourse.bass as bass
import concourse.mybir as mybir
import concourse.tile as tile

B, L, M, P = 128, 16, 32, 4
NCORES = 8
BLOC = B // NCORES  # 16 batch elements per core
J = M * P  # 128
EPS = 1e-20
REG_C = 0.01
REG_THETA_M = 0.01
REG_THETA_P = 0.01

F32 = mybir.dt.float32
BF16 = mybir.dt.bfloat16
MM_DT = mybir.dt.float32r
AF = mybir.ActivationFunctionType
ALU = mybir.AluOpType

CHUNKS = [int(x) for x in os.environ.get("MANTIS_CHUNKS", "2,4,5,5").split(",")]
# batch ids whose norm square+reduce runs on DVE (rest on ACT)
DVE_SQ_IDS = [
    int(x) for x in os.environ.get("MANTIS_DVE_SQ", "14,15").split(",") if x
]
# square mode for the DVE ids: custom (fused sq+accum), dsq (copy+stt)
SQ_MODE = os.environ.get("MANTIS_SQ_MODE", "custom")
# which group-g L3/L1/L2 builds go to Pool: "g1" (default), "none"
POOL_MODE = os.environ.get("MANTIS_POOL", "none")
POOL_CH = {int(x) for x in os.environ.get("MANTIS_POOL_CH", "").split(",") if x}

# params column layout
PC_THETA = 0  # 16 cols: theta_t[j, l]
PC_COEF = 16  # 1 col
PC_SCAL = 17  # 1 col: pi / 2^(p(j)+1)
PC_DVEC = 18  # 1 col: 1/n for the var terms (rows 0:37)
PC_HALFPI = 19  # 1 col: pi/2 (ACT bias for cos-via-sin)
PC_MASK = 20  # 37 cols: [ones | mask_p(4) | mask_m(32)]
PC_REGW = 57  # 17 cols: reg weights (rows 0:37)
PC_INDS = 74  # BLOC*L cols: input_ds slice (broadcast over partitions)
P_COLS = 74 + BLOC * L  # 330

# FIN column layout ([128, 32])
FC_SQ = 0  # 16: per-b norm partials (per partition = T1 row)
FC_AMP = 16  # 16: per-b amp partials (per partition = j)

_SQ_OP = None


def _register_sq_reduce():
    """Register a custom DVE op: out = sq(in0), accum_out = s0 + sum(out).
    Reads PSUM once -> legal fused square+reduce of the D matrix."""
    global _SQ_OP
    if _SQ_OP is not None:
        return _SQ_OP
    import concourse.dve_ops as dops
    from concourse.dve_spec import Spec, Src0, C0, sq, lower
    from concourse.dve_uop import DveOpSpec
    from operator import add

    name = "SQ_REDUCE_ANT"
    for op in dops.OPS:
        if op.name == name:
            _SQ_OP = op
            return op

    def _ref(in0, in1, s0, s1, imm2):
        b = (in0.astype(np.float32) ** 2).astype(np.float32)
        return b, s0 + b.reshape(b.shape[0], -1).sum(-1, keepdims=True)

    spec = Spec(body=sq(Src0), accum=add, accum_init=C0, reference=_ref)
    row = max(dops._SUB_OPCODE_FOR_NAME.values()) + 1
    shas = {}
    for ver in ("v3", "v4"):
        try:
            tmp = DveOpSpec(name=name, opcode=row, uops=lower(spec, ver=ver), rd1_en=False)
            shas[ver] = tmp.sha(ver)
        except Exception:
            pass
    op = dops.DveOp(name, spec, subdim=False, uops_sha=shas)
    dops.OPS.append(op)
    dops.CUSTOM_DVE_SPECS[name] = spec
    dops._SUB_OPCODE_FOR_NAME[name] = row
    _SQ_OP = op
    return op


def build_params() -> np.ndarray:
    pr = np.zeros((J, P_COLS), dtype=np.float32)
    sf = (np.pi / 2.0 ** (np.arange(P) + 1.0)).astype(np.float32)
    pr[:, PC_SCAL] = np.tile(sf, M)
    # dvec: 1/n divisors for var terms
    pr[0, PC_DVEC] = 1.0 / 128.0
    pr[1:5, PC_DVEC] = 1.0 / 32.0
    pr[5:37, PC_DVEC] = 1.0 / 4.0
    # masks
    pr[:, PC_MASK] = 1.0  # ones
    jj = np.arange(J)
    pr[:, PC_MASK + 1 : PC_MASK + 37] = 0.0
    pr[jj, PC_MASK + 1 + (jj % 4)] = 1.0  # mask_p
    pr[jj, PC_MASK + 5 + (jj // 4)] = 1.0  # mask_m
    # REGW (rows 0:37): weight for each cell of (S^2/n - SS) so that
    # sum(REGW * (S^2/n - SS)) == reg_total.  var = (SS - S^2/n)/(n-1), so
    # weight = -reg_coeff * mean_factor / (n-1).
    pr[0, PC_REGW + 16] = -REG_C / 127.0
    pr[1:5, PC_REGW : PC_REGW + 16] = -REG_THETA_M / 64.0 / 31.0
    pr[5:37, PC_REGW : PC_REGW + 16] = -REG_THETA_P / 512.0 / 3.0
    pr[:, PC_HALFPI] = np.pi / 2.0
    return pr


def build_program():
    """Build the SPMD Bass/Tile program (identical on all 8 cores)."""
    if SQ_MODE == "custom":
        _register_sq_reduce()
    nc = bacc.Bacc(
        "TRN2",
        target_bir_lowering=False,
        debug=False,
        num_devices=NCORES,
    )
    params_d = nc.dram_tensor("params", [J, P_COLS], F32, kind="ExternalInput")
    out_d = nc.dram_tensor("out", [1, 2], F32, kind="ExternalOutput")

    with tile.TileContext(nc) as tc:
        with (
            tc.tile_pool(name="const", bufs=1) as cpool,
            tc.tile_pool(name="work", bufs=1) as wpool,
            tc.tile_pool(name="dps", bufs=5, space=bass.MemorySpace.PSUM) as dpool,
            tc.tile_pool(name="fps", bufs=1, space=bass.MemorySpace.PSUM) as fpool,
        ):
            _emit(nc, tc, cpool, wpool, dpool, fpool, params_d, out_d)
    nc.compile()
    return nc


def _emit(nc, tc, cpool, wpool, dpool, fpool, params_d, out_d):
    params = cpool.tile([J, P_COLS], F32, tag="params")
    nc.sync.dma_start(params[:], params_d[:, :])

    theta_ap = params[:, PC_THETA : PC_THETA + L]
    coef_ap = params[:, PC_COEF : PC_COEF + 1]
    scal_ap = params[:, PC_SCAL : PC_SCAL + 1]
    inds_ap = params[:, PC_INDS : PC_INDS + BLOC * L]

    # --- stage A: ARG[j, (i,l)] = theta[j,l] + scal[j]*inds[i,l]
    arg = wpool.tile([J, BLOC * L], F32, tag="arg")
    in_bc = inds_ap.rearrange("j (i l) -> j i l", i=BLOC, l=L)
    th_bc = theta_ap.unsqueeze(1).broadcast_to([J, BLOC, L])
    arg_v = arg[:].rearrange("j (i l) -> j i l", i=BLOC, l=L)
    nc.vector.scalar_tensor_tensor(
        out=arg_v, in0=in_bc, scalar=scal_ap, in1=th_bc,
        op0=ALU.mult, op1=ALU.add,
    )

    # --- CS[j, (t,i,l)]: t=0 -> cos(A), t=1 -> sin(A); bf16 out
    # cos(A) = sin(pi/2 - A); A in (-1, 2.58) keeps both args in [-pi, pi].
    cs = wpool.tile([J, 2 * BLOC * L], F32, tag="cs")
    nc.scalar.activation(
        cs[:, 0 : BLOC * L], arg[:], AF.Sin,
        bias=params[:, PC_HALFPI : PC_HALFPI + 1], scale=-1.0,
    )
    nc.scalar.activation(cs[:, BLOC * L : 2 * BLOC * L], arg[:], AF.Sin)

    # fold coef into site l=0 (both branches) => every T1 combo of group 0
    # carries exactly one coef_j factor.
    cs_v = cs[:].rearrange("j (t i l) -> j t i l", t=2, i=BLOC, l=L)
    nc.vector.tensor_scalar_mul(cs_v[:, :, :, 0:1], cs_v[:, :, :, 0:1], coef_ap)

    final = wpool.tile([1, 2], F32, tag="final")

    # --- doubling: L1 (site pairs, 4 combos), L2 (quads, 16 combos)
    # group 0 on DVE, group 1 on Pool (own SBUF port; overlaps DVE).
    eng = {0: nc.vector, 1: (nc.gpsimd if POOL_MODE == "g1" else nc.vector)}
    l1 = [wpool.tile([J, BLOC * 16], F32, tag=f"l1_{g}", name=f"l1_{g}") for g in range(2)]
    l2 = [wpool.tile([J, BLOC * 32], F32, tag=f"l2_{g}", name=f"l2_{g}") for g in range(2)]
    for g in range(2):
        lo = g * 8  # first site of the group
        o1all = l1[g][:].rearrange(
            "j (i s t1 t2) -> j i s t1 t2", i=BLOC, s=4, t1=2, t2=2
        )
        for t1 in range(2):
            in1 = (
                cs_v[:, t1, :, lo : lo + 8 : 2]
                .unsqueeze(3)
                .broadcast_to([J, BLOC, 4, 2])
            )
            in2 = cs_v[:, :, :, lo + 1 : lo + 8 : 2].transpose([0, 2, 3, 1])
            o1 = o1all[:, :, :, t1, :]
            eng[g].tensor_tensor(out=o1, in0=in1, in1=in2, op=ALU.mult)
        l1v = l1[g][:].rearrange("j (i s c) -> j i s c", i=BLOC, s=4, c=4)
        o2all = l2[g][:].rearrange(
            "j (i d q1 q2) -> j i d q1 q2", i=BLOC, d=2, q1=4, q2=4
        )
        for d in range(2):
            in1 = l1v[:, :, 2 * d, :].unsqueeze(3).broadcast_to([J, BLOC, 4, 4])
            in2 = l1v[:, :, 2 * d + 1, :].unsqueeze(2).broadcast_to([J, BLOC, 4, 4])
            o2 = o2all[:, :, d, :, :]
            eng[g].tensor_tensor(out=o2, in0=in1, in1=in2, op=ALU.mult)

    # =====================================================================
    # regularization path -- depends only on params; runs early, fully
    # overlapped with the heavy math.
    fin_r = wpool.tile([J, 34], F32, tag="fin_r")
    nc.vector.tensor_copy(fin_r[:, 0:17], params[:, 0:17])
    nc.vector.tensor_tensor(out=fin_r[:, 17:34], in0=params[:, 0:17],
                        in1=params[:, 0:17], op=ALU.mult)
    fout_r = fpool.tile([37, 34], F32, tag="fout_r")
    nc.tensor.matmul(fout_r[:], params[:, PC_MASK : PC_MASK + 37], fin_r[:])
    ss_part = fout_r[0:37, 17:34]
    sv = wpool.tile([37, 17], F32, tag="sv")
    nc.vector.tensor_copy(sv[:], fout_r[0:37, 0:17])
    v1 = wpool.tile([37, 17], F32, tag="v1")
    nc.vector.tensor_tensor(out=v1[:], in0=sv[:], in1=sv[:], op=ALU.mult)
    v2 = wpool.tile([37, 17], F32, tag="v2")
    nc.vector.scalar_tensor_tensor(
        out=v2[:], in0=v1[:],
        scalar=params[0:37, PC_DVEC : PC_DVEC + 1],
        in1=ss_part, op0=ALU.mult, op1=ALU.subtract,
    )
    v3 = wpool.tile([37, 17], F32, tag="v3")
    nc.vector.tensor_tensor(
        out=v3[:], in0=v2[:],
        in1=params[0:37, PC_REGW : PC_REGW + 17], op=ALU.mult,
    )
    v4 = wpool.tile([37, 17], F32, tag="v4")
    v5 = wpool.tile([37, 1], F32, tag="v5")
    nc.vector.tensor_scalar(
        out=v4[:], in0=v3[:], scalar1=1.0, scalar2=None,
        op0=ALU.mult, op1=ALU.add, accum_out=v5[:],
    )
    rt = fpool.tile([1, 1], F32, tag="rt")
    nc.tensor.matmul(rt[:], params[0:37, PC_MASK : PC_MASK + 1], v5[:])
    nc.vector.tensor_copy(final[0:1, 1:2], rt[:])
    # =====================================================================

    # --- L3 chunked by batch; per-chunk U tiles so PE/ACT pipeline per chunk
    fin = wpool.tile([J, 32], F32, tag="fin")  # 0:16 sumsq, 16:32 amp
    chunk_sizes = CHUNKS
    assert sum(chunk_sizes) == BLOC
    sq_dve = set(DVE_SQ_IDS)
    sqjunk = wpool.tile([J, 512], BF16, tag="sqjunk")
    uc = [
        wpool.tile([J, BLOC * 256], MM_DT, tag=f"u_{g}", name=f"u_{g}")
        for g in range(2)
    ]
    with tc.tile_pool(name="dsqp", bufs=2) as spool:
        i0 = 0
        for c, csz in enumerate(chunk_sizes):
            for g in range(2):
                l2v = l2[g][:].rearrange(
                    "j (i d c16) -> j i d c16", i=BLOC, d=2, c16=16
                )
                in1 = (
                    l2v[:, i0 : i0 + csz, 0, :]
                    .unsqueeze(3)
                    .broadcast_to([J, csz, 16, 16])
                )
                in2 = (
                    l2v[:, i0 : i0 + csz, 1, :]
                    .unsqueeze(2)
                    .broadcast_to([J, csz, 16, 16])
                )
                ov = uc[g][:, i0 * 256 : (i0 + csz) * 256].rearrange(
                    "j (i u1 u2) -> j i u1 u2", i=csz, u1=16, u2=16
                )
                e = eng[g] if (g == 1 and c in POOL_CH) else nc.vector
                e.tensor_tensor(out=ov, in0=in1, in1=in2, op=ALU.mult)

            # D matmuls + square/accum for this chunk's batch elements
            for k in range(csz):
                i = i0 + k
                dt = dpool.tile([J, 512], F32, tag="D")
                rhs = uc[1][:, i * 256 : (i + 1) * 256]
                for h in range(2):
                    lhsT = uc[0][:, i * 256 + h * 128 : i * 256 + (h + 1) * 128]
                    nc.tensor.matmul(dt[:, h * 256 : (h + 1) * 256], lhsT, rhs)
                if i in sq_dve and SQ_MODE == "custom":
                    nc.vector._custom_dve(
                        _SQ_OP, out=sqjunk[:], in0=dt[:],
                        accum_out=fin[:, i : i + 1],
                    )
                elif i in sq_dve:
                    dsq = spool.tile([J, 512], F32, tag="dsq", name="dsq")
                    nc.vector.tensor_copy(dsq[:], dt[:])
                    nc.vector.scalar_tensor_tensor(
                        out=dsq[:], in0=dsq[:], scalar=1.0, in1=dsq[:],
                        op0=ALU.mult, op1=ALU.mult,
                        accum_out=fin[:, i : i + 1],
                    )
                else:
                    nc.scalar.activation(
                        dt[:], dt[:], AF.Square,
                        accum_out=fin[:, i : i + 1],
                    )
            i0 += csz

    # amp partials, all 16 b in one instr: fin[:, 16+i] = cU1[j,i,0]*U2[j,i,0]
    u1v = uc[0][:].rearrange("j (i t) -> j i t", i=BLOC, t=256)
    u2v = uc[1][:].rearrange("j (i t) -> j i t", i=BLOC, t=256)
    nc.vector.tensor_tensor(
        out=fin[:, 16:32], in0=u1v[:, :, 0], in1=u2v[:, :, 0], op=ALU.mult,
    )

    # --- loss tail: one ones-matmul + short scalar chain
    fout = fpool.tile([1, 32], F32, tag="fout")
    nc.tensor.matmul(fout[:], params[:, PC_MASK : PC_MASK + 1], fin[:])
    # g0 = [r(16) | amp(16)] in SBUF
    g0 = wpool.tile([1, 32], F32, tag="g0")
    nc.vector.tensor_copy(g0[:], fout[0:1, 0:32])
    m2 = wpool.tile([1, BLOC], F32, tag="m2")
    nc.vector.tensor_tensor(
        out=m2[:], in0=g0[0:1, 16:32], in1=g0[0:1, 16:32], op=ALU.mult
    )
    # lno = [ln(amp^2 + EPS*r) (16) | ln(r) (16)] via two independent Lns
    tt = wpool.tile([1, 16], F32, tag="tt")
    nc.vector.scalar_tensor_tensor(
        out=tt[:], in0=g0[0:1, 0:16], scalar=EPS, in1=m2[:],
        op0=ALU.mult, op1=ALU.add,
    )
    lno = wpool.tile([1, 32], F32, tag="lno")
    nc.scalar.activation(lno[0:1, 16:32], g0[0:1, 0:16], AF.Ln)
    nc.scalar.activation(lno[0:1, 0:16], tt[:], AF.Ln)
    diff = wpool.tile([1, BLOC], F32, tag="diff")
    nc.vector.tensor_tensor(
        out=diff[:], in0=lno[0:1, 0:16], in1=lno[0:1, 16:32], op=ALU.subtract
    )
    scr6 = wpool.tile([1, BLOC], F32, tag="scr6")
    nc.vector.tensor_scalar(
        out=scr6[:], in0=diff[:], scalar1=-1.0 / float(B), scalar2=None,
        op0=ALU.mult, op1=ALU.add, accum_out=final[0:1, 0:1],
    )

    nc.sync.dma_start(out_d[:, :], final[:])


def make_in_maps(input_ds, theta, coef):
    input_ds = np.asarray(input_ds, dtype=np.float32)
    theta = np.asarray(theta, dtype=np.float32)
    coef = np.asarray(coef, dtype=np.float32)
    pr = build_params()
    pr[:, PC_THETA : PC_THETA + L] = theta.transpose(1, 2, 0).reshape(J, L)
    pr[:, PC_COEF] = coef.reshape(J)
    in_maps = []
    for c in range(NCORES):
        prc = pr.copy()
        sl = input_ds[c * BLOC : (c + 1) * BLOC, :].reshape(1, BLOC * L)
        prc[:, PC_INDS:] = np.broadcast_to(sl, (J, BLOC * L))
        in_maps.append({"params": prc})
    return in_maps


_NC_CACHE = None


def _get_program():
    global _NC_CACHE
    if _NC_CACHE is None:
        _NC_CACHE = build_program()
    return _NC_CACHE


def combine_outputs(results):
    loss = 0.0
    for c in range(NCORES):
        loss += float(results[c]["out"][0, 0])
    loss += float(results[0]["out"][0, 1])
    return np.float32(loss)


def kernel(input_ds, theta, coef):
    from concourse.bass_utils import run_bass_kernel_spmd

    nc = _get_program()
    in_maps = make_in_maps(input_ds, theta, coef)
    res = run_bass_kernel_spmd(nc, in_maps, core_ids=list(range(NCORES)))
    return combine_outputs(res.results)
